# revision 1
# baseline (speedup 1.0000x reference)
# Trainium2 Bass kernel for nn_Decoder (LSTM decoder + GCN message passing).
#
# Strategy (8 NeuronCores, SPMD):
#   * Data-parallel over nodes N=10000 -> 1250 nodes/core for fc2 + LSTM +
#     projection. State kept feature-major ([H, nodes]) so every matmul is
#     PE-friendly with K=H=128 and no transposes.
#   * Algebraic rewrite: the GCN aggregation and fc3 are both linear, so
#     aggregate AFTER projecting features to NF=16:
#        x_hat[n,t] = dinv[n] * sum_{e: dst=n} (dinv[src] * mask[src] * hs[t,src] @ (W_gcn@W_fc3))
#                     + (b_gcn@W_fc3 + b_fc3)
#     This shrinks the scatter/gather payload 8x (H=128 -> NF=16 per t).
#   * Y table ([N, T*NF], fp16, dinv*mask pre-scaled) is AllGather'ed across
#     the 8 cores. The scatter-add over edges is reformulated as a
#     block-sparse matmul: the host densifies the normalized adjacency into
#     128x128 blocks A[dst_tile, src_blk] (entry = edge multiplicity), and
#     each core computes agg[dst_tile] = sum_sb A[dst_tile,sb].T @ Y[sb]
#     streaming Y sequentially (direct DMA only - indirect DMA's per-row
#     descriptor generation on GpSimd would cost ~200us).
import os
import numpy as np

import concourse.bass as bass
import concourse.bacc as bacc
import concourse.tile as tile
from concourse import mybir
from concourse import bass_utils

P = 128
N, T, NF, H, L, E = 10000, 12, 16, 128, 64, 160000
NCORES = 8
NCN = N // NCORES            # 1250 nodes per core
NTILES = (NCN + P - 1) // P  # 10 dst tiles per core
NSB = (N + P - 1) // P       # 79 source blocks (last has 16 rows)
CH = [(0, 512), (512, 512), (1024, NCN - 1024)]  # LSTM node chunks (<=512)
TNF = T * NF                 # 192

F32 = mybir.dt.float32
F16 = mybir.dt.float16
U8 = mybir.dt.uint8

# gate q: 0=i, 1=f, 2=g, 3=o ; activation: sigmoid for i,f,o ; tanh for g
GATE_FUNCS = ["Sigmoid", "Sigmoid", "Tanh", "Sigmoid"]

_BUILD_CACHE = {}
LAST_RESULTS = None  # BassKernelResults of the most recent run (for test harness)


def _build():
    nc = bacc.Bacc("TRN2", target_bir_lowering=False, debug=False,
                   num_devices=NCORES)

    # ---------------- I/O declarations ----------------
    zT = nc.dram_tensor("zT", [L, NCN], F32, kind="ExternalInput")
    xm = nc.dram_tensor("xm", [NCN, TNF], U8, kind="ExternalInput")
    wfc2 = nc.dram_tensor("wfc2", [L, H], F32, kind="ExternalInput")
    b2 = nc.dram_tensor("b2", [P, 1], F32, kind="ExternalInput")
    wih = nc.dram_tensor("wih", [H, 4 * H], F16, kind="ExternalInput")
    whh = nc.dram_tensor("whh", [H, 4 * H], F16, kind="ExternalInput")
    bg = nc.dram_tensor("bg", [P, 4], F32, kind="ExternalInput")
    wcomb = nc.dram_tensor("wcomb", [H, NF], F16, kind="ExternalInput")
    bout = nc.dram_tensor("bout", [P, TNF], F32, kind="ExternalInput")
    dinvt = nc.dram_tensor("dinvt", [P, NTILES], F32, kind="ExternalInput")
    # A-blocks, wave-major: row (w*NSB + sb)*128 + p ; col = k_local*128 + drel
    ablk = nc.dram_tensor("ablk", [2 * NSB * P, 5 * P], F16,
                          kind="ExternalInput")
    xhat = nc.dram_tensor("xhat", [NCN, TNF], F32, kind="ExternalOutput")

    with tile.TileContext(nc) as tc:
        with tc.tile_pool(name="cpool", bufs=1) as cp, \
             tc.tile_pool(name="spool", bufs=1) as sp, \
             tc.tile_pool(name="dram", bufs=1, space="DRAM") as dp:

            # ---- constant loads ----
            zt_sb = cp.tile([L, NCN], F32)
            nc.sync.dma_start(zt_sb[:], zT[:])
            wfc2_sb = cp.tile([L, H], F32)
            nc.sync.dma_start(wfc2_sb[:], wfc2[:])
            b2_sb = cp.tile([P, 1], F32)
            nc.sync.dma_start(b2_sb[:], b2[:])
            wih_sb = cp.tile([H, 4 * H], F16)
            nc.sync.dma_start(wih_sb[:], wih[:])
            whh_sb = cp.tile([H, 4 * H], F16)
            nc.sync.dma_start(whh_sb[:], whh[:])
            bg_sb = cp.tile([P, 4], F32)
            nc.sync.dma_start(bg_sb[:], bg[:])
            wcomb_sb = cp.tile([H, NF], F16)
            nc.sync.dma_start(wcomb_sb[:], wcomb[:])
            bout_sb = cp.tile([P, TNF], F32)
            nc.sync.dma_start(bout_sb[:], bout[:])
            dinv_sb = cp.tile([P, NTILES], F32)
            nc.sync.dma_start(dinv_sb[:], dinvt[:])

            SL = [(0, 96), (96, 48), (144, 48)]  # (col0, width) per AG slice
            yshard_s = [dp.tile([NCN, w], F16, name=f"yshard{i}")
                        for i, (c0, w) in enumerate(SL)]
            yfull_s = [dp.tile([N, w], F16, addr_space="Shared",
                               name=f"yfull{i}")
                       for i, (c0, w) in enumerate(SL)]

            # ---- node mask * dinv (per node-block) ----
            mdv_sb = sp.tile([P, NTILES], F32)
            with tc.tile_pool(name="wp0", bufs=3) as wp0:
                for k in range(NTILES):
                    rows = min(P, NCN - k * P)
                    xmu = wp0.tile([P, TNF], U8, tag="xmu", bufs=3)
                    nc.sync.dma_start(xmu[:rows], xm[k * P:k * P + rows, :])
                    xmf = wp0.tile([P, TNF], F32, tag="xmf", bufs=3)
                    nc.vector.tensor_copy(out=xmf[:rows], in_=xmu[:rows])
                    mx = wp0.tile([P, 1], F32, tag="mx", bufs=3)
                    nc.vector.reduce_max(out=mx[:rows], in_=xmf[:rows],
                                         axis=mybir.AxisListType.X)
                    nc.vector.tensor_mul(out=mdv_sb[:rows, k:k + 1],
                                         in0=mx[:rows],
                                         in1=dinv_sb[:rows, k:k + 1])

            # ---- hd = z @ W_fc2 + b_fc2 (feature-major: hdT [H, nodes]) ----
            hdT = sp.tile([H, NCN], F16)
            with tc.tile_pool(name="psI", bufs=2, space="PSUM") as psI:
                for off, sz in CH:
                    ph = psI.tile([P, 512], F32, tag="ph", bufs=2)
                    nc.tensor.matmul(out=ph[:, :sz], lhsT=wfc2_sb[:],
                                     rhs=zt_sb[:, off:off + sz],
                                     start=True, stop=True)
                    nc.scalar.activation(
                        out=hdT[:, off:off + sz], in_=ph[:, :sz],
                        func=mybir.ActivationFunctionType.Identity,
                        bias=b2_sb[:, :1])

            # ---- LSTM (T steps, feature-major state) ----
            # Full-width (1250) PSUM per gate; weight loads ordered so each
            # of the 8 weight tiles is loaded once per step.
            cstate = sp.tile([P, NCN], F32)
            nc.vector.memset(cstate[:], 0.0)

            hs = []  # hs[t] tiles [H, NCN]
            hs_pool = tc.tile_pool(name="hspool", bufs=1)
            hsp = hs_pool.__enter__()
            ysb_t = [sp.tile([P, TNF], F16, name=f"ysb_{k}", tag=f"ysb_{k}")
                     for k in range(NTILES)]
            NFULL = NSB - 1  # 78 full source blocks, then a 16-row tail
            ytab = sp.tile([P, NSB * TNF], F16, name="ytab")

            def ship_slice(i):
                c0, w = SL[i]
                for k in range(NTILES):
                    rows = min(P, NCN - k * P)
                    nc.sync.dma_start(yshard_s[i][k * P:k * P + rows, :],
                                      ysb_t[k][:rows, c0:c0 + w])
                nc.gpsimd.collective_compute(
                    "AllGather", mybir.AluOpType.bypass,
                    replica_groups=[list(range(NCORES))],
                    ins=[yshard_s[i].opt()], outs=[yfull_s[i].opt()],
                )
                nc.sync.dma_start(
                    ytab[:, :NFULL * TNF].rearrange(
                        "p (sb f) -> p sb f", f=TNF)[:, :, c0:c0 + w],
                    yfull_s[i][:NFULL * P, :].rearrange(
                        "(sb p) f -> p sb f", p=P))
                nc.sync.dma_start(
                    ytab[:N - NFULL * P,
                         NFULL * TNF + c0:NFULL * TNF + c0 + w],
                    yfull_s[i][NFULL * P:, :])
            with tc.tile_pool(name="psG", bufs=2, space="PSUM") as psG, \
                 tc.tile_pool(name="psY", bufs=2, space="PSUM") as psY, \
                 tc.tile_pool(name="wpL", bufs=2) as wpL:
                def emit_proj(t):
                    for k in range(NTILES):
                        rows = min(P, NCN - k * P)
                        py = psY.tile([P, NF], F32, tag="py", bufs=2)
                        nc.tensor.matmul(out=py[:rows, :],
                                         lhsT=hs[t][:, k * P:k * P + rows],
                                         rhs=wcomb_sb[:],
                                         start=True, stop=True)
                        nc.vector.tensor_scalar(
                            out=ysb_t[k][:rows, t * NF:(t + 1) * NF],
                            in0=py[:rows, :],
                            scalar1=mdv_sb[:rows, k:k + 1],
                            scalar2=None, op0=mybir.AluOpType.mult)
                    if t == 5:
                        ship_slice(0)
                    elif t == 8:
                        ship_slice(1)

                for t in range(T):
                    prev = hdT if t == 0 else hs[t - 1]
                    sg = [None] * 4
                    pqs = [None] * 4

                    def emit_ih(q):
                        wsl = slice(q * H, (q + 1) * H)
                        pqs[q] = psG.tile([P, NCN], F32, name="pq", tag="pq", bufs=2)
                        for off, sz in CH:
                            nc.tensor.matmul(out=pqs[q][:, off:off + sz],
                                             lhsT=wih_sb[:, wsl],
                                             rhs=hdT[:, off:off + sz],
                                             start=True, stop=False)

                    def emit_hh_act(q):
                        wsl = slice(q * H, (q + 1) * H)
                        for off, sz in CH:
                            nc.tensor.matmul(out=pqs[q][:, off:off + sz],
                                             lhsT=whh_sb[:, wsl],
                                             rhs=prev[:, off:off + sz],
                                             start=False, stop=True)
                        sg[q] = wpL.tile([P, NCN], F32, name=f"sg{q}", tag=f"sg{q}", bufs=2)
                        nc.scalar.activation(
                            out=sg[q][:], in_=pqs[q][:],
                            func=getattr(mybir.ActivationFunctionType,
                                         GATE_FUNCS[q]),
                            bias=bg_sb[:, q:q + 1])

                    emit_ih(0)
                    emit_ih(1)
                    if t > 0:
                        emit_proj(t - 1)   # fills PE while h_{t-1} finishes
                    emit_hh_act(0)
                    emit_hh_act(1)
                    for q in (2, 3):
                        emit_ih(q)
                        emit_hh_act(q)

                    nc.vector.tensor_mul(out=cstate[:], in0=cstate[:],
                                         in1=sg[1][:])
                    tmp = wpL.tile([P, NCN], F32, tag="tmp", bufs=2)
                    nc.vector.tensor_mul(out=tmp[:], in0=sg[0][:], in1=sg[2][:])
                    nc.vector.tensor_add(out=cstate[:], in0=cstate[:],
                                         in1=tmp[:])
                    thc = wpL.tile([P, NCN], F32, tag="thc", bufs=2)
                    nc.scalar.activation(
                        out=thc[:], in_=cstate[:],
                        func=mybir.ActivationFunctionType.Tanh)
                    h_t = hsp.tile([P, NCN], F16, name=f"h_{t}", tag=f"h_{t}")
                    nc.vector.tensor_mul(out=h_t[:], in0=sg[3][:], in1=thc[:])
                    hs.append(h_t)
                emit_proj(T - 1)
                ship_slice(2)
            hs_pool.__exit__(None, None, None)  # release hs SBUF before GCN

            # ---- GCN aggregation: agg[k] = sum_sb A[k,sb].T @ Y[sb] ----
            # Whole Y table SBUF-resident: ytab[p, sb*192+f] = Y[sb*128+p, f].
            # A-blocks stream in 8-sb chunks. 2 waves of 5 dst tiles.
            with tc.tile_pool(name="psC", bufs=1, space="PSUM") as psC, \
                 tc.tile_pool(name="wpC", bufs=2) as wpC:
                CHUNK = 8
                sb_chunks = [(s, min(s + CHUNK, NSB))
                             for s in range(0, NSB, CHUNK)]
                for w, wave in enumerate((range(0, 5), range(5, NTILES))):
                    wave = list(wave)
                    pa = {k: psC.tile([P, TNF], F32, name=f"pa_{k}",
                                      tag=f"pa{i}", bufs=1)
                          for i, k in enumerate(wave)}
                    for (s0, s1) in sb_chunks:
                        nsb_c = s1 - s0
                        abc = wpC.tile([P, CHUNK * 5 * P], F16, tag="abc",
                                       bufs=3)
                        r0 = (w * NSB + s0) * P
                        r1 = (w * NSB + s1) * P
                        nc.gpsimd.dma_start(
                            abc[:, :nsb_c * 5 * P].rearrange(
                                "p (sb d) -> p sb d", d=5 * P),
                            ablk[r0:r1, :].rearrange("(sb p) d -> p sb d",
                                                     p=P))
                        for sb in range(s0, s1):
                            srows = min(P, N - sb * P)
                            aoff = (sb - s0) * 5 * P
                            for i, k in enumerate(wave):
                                nc.tensor.matmul(
                                    out=pa[k][:],
                                    lhsT=abc[:srows,
                                             aoff + i * P:aoff + (i + 1) * P],
                                    rhs=ytab[:srows,
                                             sb * TNF:(sb + 1) * TNF],
                                    start=(sb == 0),
                                    stop=(sb == NSB - 1))
                    for i, k in enumerate(wave):
                        rows = min(P, NCN - k * P)
                        osb = wpC.tile([P, TNF], F32, tag="osb", bufs=2)
                        nc.vector.tensor_scalar(out=osb[:rows],
                                                in0=pa[k][:rows],
                                                scalar1=dinv_sb[:rows, k:k + 1],
                                                scalar2=None,
                                                op0=mybir.AluOpType.mult)
                        nc.vector.tensor_add(out=osb[:rows], in0=osb[:rows],
                                             in1=bout_sb[:rows])
                        nc.sync.dma_start(xhat[k * P:k * P + rows, :],
                                          osb[:rows])

    nc.compile()
    return nc


def _preprocess(z, edge_index, x_mask, W_fc2, b_fc2, W_ih, W_hh, b_ih, b_hh,
                W_gcn, b_gcn, W_fc3, b_fc3):
    z = np.asarray(z, np.float32)
    edge_index = np.asarray(edge_index).astype(np.int64)
    x_mask = np.asarray(x_mask)
    src = edge_index[0]
    dst = edge_index[1]
    deg = (np.bincount(dst, minlength=N) + 1.0)
    dinv = (1.0 / np.sqrt(deg)).astype(np.float32)

    src_all = np.concatenate([src, np.arange(N, dtype=np.int64)])
    dst_all = np.concatenate([dst, np.arange(N, dtype=np.int64)])

    # densify adjacency into per-core wave-major A blocks:
    # row (w*NSB + sb)*128 + srel ; col (ktile%5)*128 + drel  (w = ktile//5)
    core_of = dst_all // NCN
    ktile = (dst_all % NCN) // P
    drel = (dst_all % NCN) % P
    sblk = src_all // P
    srel = src_all % P

    a_blocks = []
    lin = (((ktile // 5) * NSB + sblk) * P + srel) * (5 * P) \
        + (ktile % 5) * P + drel
    nblk_lin = 2 * NSB * P * 5 * P
    for c in range(NCORES):
        m = core_of == c
        counts = np.bincount(lin[m], minlength=nblk_lin)
        a_blocks.append(counts.astype(np.float16).reshape(2 * NSB * P, 5 * P))

    Wcomb = np.ascontiguousarray((np.asarray(W_gcn, np.float32)
                                  @ np.asarray(W_fc3, np.float32))
                                 .astype(np.float16))
    bias16 = (np.asarray(b_gcn, np.float32) @ np.asarray(W_fc3, np.float32)
              + np.asarray(b_fc3, np.float32))
    bout_t = np.ascontiguousarray(np.tile(bias16, (P, T)).astype(np.float32))
    bgv = (np.asarray(b_ih, np.float32) + np.asarray(b_hh, np.float32))
    bg_t = np.ascontiguousarray(bgv.reshape(4, P).T.astype(np.float32))
    b2_t = np.ascontiguousarray(np.asarray(b_fc2, np.float32).reshape(P, 1))
    wih_t = np.ascontiguousarray(np.asarray(W_ih, np.float32).T.astype(np.float16))
    whh_t = np.ascontiguousarray(np.asarray(W_hh, np.float32).T.astype(np.float16))
    wfc2_t = np.ascontiguousarray(np.asarray(W_fc2, np.float32))

    in_maps = []
    for c in range(NCORES):
        sl = slice(c * NCN, (c + 1) * NCN)
        dv = dinv[sl]
        dinv_t = np.zeros((P, NTILES), np.float32)
        for k in range(NTILES):
            rows = min(P, NCN - k * P)
            dinv_t[:rows, k] = dv[k * P:k * P + rows]
        in_maps.append({
            "zT": np.ascontiguousarray(z[sl].T),
            "xm": np.ascontiguousarray(
                x_mask[sl].reshape(NCN, TNF).astype(np.uint8)),
            "wfc2": wfc2_t,
            "b2": b2_t,
            "wih": wih_t,
            "whh": whh_t,
            "bg": bg_t,
            "wcomb": Wcomb,
            "bout": bout_t,
            "dinvt": dinv_t,
            "ablk": a_blocks[c],
        })
    return in_maps


def kernel(z, edge_index, x_mask, W_fc2, b_fc2, W_ih, W_hh, b_ih, b_hh,
           W_gcn, b_gcn, W_fc3, b_fc3):
    global LAST_RESULTS
    in_maps = _preprocess(z, edge_index, x_mask, W_fc2, b_fc2,
                          W_ih, W_hh, b_ih, b_hh,
                          W_gcn, b_gcn, W_fc3, b_fc3)
    if "nc" not in _BUILD_CACHE:
        _BUILD_CACHE["nc"] = _build()
    nc = _BUILD_CACHE["nc"]

    trace = bool(int(os.environ.get("KERNEL_TRACE", "0")))
    res = bass_utils.run_bass_kernel_spmd(
        nc, in_maps, core_ids=list(range(NCORES)), trace=trace)
    LAST_RESULTS = res

    out = np.empty((N, T, NF), np.float32)
    for c in range(NCORES):
        out[c * NCN:(c + 1) * NCN] = res.results[c]["xhat"].reshape(NCN, T, NF)
    return out



# revision 7
# speedup vs baseline: 1.1369x; 1.1369x over previous
# Trainium2 Bass kernel for nn_Decoder (LSTM decoder + GCN message passing).
#
# Strategy (8 NeuronCores, SPMD):
#   * Data-parallel over nodes N=10000 -> 1250 nodes/core for fc2 + LSTM +
#     projection. State kept feature-major ([H, nodes]) so every matmul is
#     PE-friendly with K=H=128 and no transposes.
#   * Algebraic rewrite: GCN aggregation and fc3 are both linear, so
#     aggregate AFTER projecting features to NF=16 (payload 8x smaller):
#        x_hat[n,t] = dinv[n] * sum_{e: dst=n} (dinv[s]*mask[s] * hs[t,s] @ (W_gcn@W_fc3))
#                     + (b_gcn@W_fc3 + b_fc3)
#   * The scatter-add over edges is a block-dense matmul: the host densifies
#     the normalized adjacency into 128x128 blocks (entry = multiplicity),
#     stored fp8-e3m4 (exact for small ints, halves HBM traffic); the PE
#     contracts them against the fp16 Y table (mixed-dtype matmul).
#   * Y table is built k-major so the AllGather input is one contiguous
#     store; a single AG at the end + 8 contiguous ytab loads (per-source-
#     core granularity) replaces the baseline's fragmented exchange.
#   * mask*dinv is computed on host (it only depends on inputs).
import os
import numpy as np
import ml_dtypes

import concourse.bass as bass
import concourse.bacc as bacc
import concourse.tile as tile
from concourse import mybir
from concourse import bass_utils

P = 128
N, T, NF, H, L, E = 10000, 12, 16, 128, 64, 160000
NCORES = 8
NCN = N // NCORES            # 1250 nodes per core
NTILES = (NCN + P - 1) // P  # 10 dst tiles per core (last has 98 rows)
NSB = NCORES * NTILES        # 80 source blocks (per-core-local blocking)
CH = [(0, 512), (512, 512), (1024, NCN - 1024)]  # matmul node chunks (<=512)
TNF = T * NF                 # 192
DCOL = NTILES * P            # 1280 A-block dst columns per core
CH_SB = 4                    # source blocks per streamed A chunk
ABUFS = 6                    # in-flight A chunks (SBUF prefetch depth)

F32 = mybir.dt.float32
F16 = mybir.dt.float16
BF16 = mybir.dt.bfloat16
F8E3 = mybir.dt.float8e3

# gate order within W_ih/W_hh: 0=i, 1=f, 2=g, 3=o
GATE_FUNCS = ["Sigmoid", "Sigmoid", "Tanh", "Sigmoid"]
GATE_ORDER = [1, 0, 2, 3]  # emit f first: starts the ACT/DVE chain earliest

_BUILD_CACHE = {}
LAST_RESULTS = None  # BassKernelResults of the most recent run (for test harness)


def _build():
    nc = bacc.Bacc("TRN2", target_bir_lowering=False, debug=False,
                   num_devices=NCORES)

    # ---------------- I/O declarations ----------------
    zT = nc.dram_tensor("zT", [L, NCN], BF16, kind="ExternalInput")
    wfc2 = nc.dram_tensor("wfc2", [L, H], BF16, kind="ExternalInput")
    b2 = nc.dram_tensor("b2", [P, 1], F32, kind="ExternalInput")
    wih = nc.dram_tensor("wih", [H, 4 * H], F16, kind="ExternalInput")
    whh = nc.dram_tensor("whh", [H, 4 * H], F16, kind="ExternalInput")
    bg = nc.dram_tensor("bg", [P, 4], F32, kind="ExternalInput")
    wcomb = nc.dram_tensor("wcomb", [H, NF], F16, kind="ExternalInput")
    bout = nc.dram_tensor("bout", [P, TNF], F32, kind="ExternalInput")
    dinvt = nc.dram_tensor("dinvt", [P, NTILES], F32, kind="ExternalInput")
    mdvt = nc.dram_tensor("mdvt", [P, NTILES], F32, kind="ExternalInput")
    # A-blocks: row sb*128 + src_rel ; col k*128 + dst_rel (sb = c*10 + k_src)
    ablk = nc.dram_tensor("ablk", [NSB * P, DCOL], F8E3, kind="ExternalInput")
    xhat = nc.dram_tensor("xhat", [NCN, TNF], F32, kind="ExternalOutput")

    with tile.TileContext(nc) as tc:
        with tc.tile_pool(name="cpool", bufs=1) as cp, \
             tc.tile_pool(name="spool", bufs=1) as sp, \
             tc.tile_pool(name="dram", bufs=1, space="DRAM") as dp:

            # ---- constant loads ----
            zt_sb = cp.tile([L, NCN], BF16)
            nc.sync.dma_start(zt_sb[:], zT[:])
            wfc2_sb = cp.tile([L, H], BF16)
            nc.sync.dma_start(wfc2_sb[:], wfc2[:])
            b2_sb = cp.tile([P, 1], F32)
            nc.sync.dma_start(b2_sb[:], b2[:])
            wih_sb = cp.tile([H, 4 * H], F16)
            nc.sync.dma_start(wih_sb[:], wih[:])
            whh_sb = cp.tile([H, 4 * H], F16)
            nc.sync.dma_start(whh_sb[:], whh[:])
            bg_sb = cp.tile([P, 4], F32)
            nc.sync.dma_start(bg_sb[:], bg[:])
            wcomb_sb = cp.tile([H, NF], F16)
            nc.sync.dma_start(wcomb_sb[:], wcomb[:])
            bout_sb = cp.tile([P, TNF], F32)
            nc.sync.dma_start(bout_sb[:], bout[:])
            dinv_sb = cp.tile([P, NTILES], F32)
            nc.sync.dma_start(dinv_sb[:], dinvt[:])
            mdv_sb = cp.tile([P, NTILES], F32)
            nc.sync.dma_start(mdv_sb[:], mdvt[:])

            # Y shard (this core's 1250 nodes), k-major: col k*192 + t*16 + f
            ysb = sp.tile([P, NTILES * TNF], F16, name="ysb")
            nc.vector.memset(ysb[:], 0.0)  # rows >= 98 of tile 9 stay zero
            yshard = dp.tile([P, NTILES * TNF], F16, name="yshard")
            yfull = dp.tile([NCORES * P, NTILES * TNF], F16,
                            addr_space="Shared", name="yfull")

            # ---- hd = z @ W_fc2 + b_fc2 (feature-major: hdT [H, nodes]) ----
            hdT = sp.tile([H, NCN], F16)
            with tc.tile_pool(name="psI", bufs=2, space="PSUM") as psI:
                for off, sz in CH:
                    ph = psI.tile([P, 512], F32, tag="ph", bufs=2)
                    nc.tensor.matmul(out=ph[:, :sz], lhsT=wfc2_sb[:],
                                     rhs=zt_sb[:, off:off + sz],
                                     start=True, stop=True)
                    nc.scalar.activation(
                        out=hdT[:, off:off + sz], in_=ph[:, :sz],
                        func=mybir.ActivationFunctionType.Identity,
                        bias=b2_sb[:, :1])

            # ---- LSTM (T steps, feature-major state) ----
            cstate = sp.tile([P, NCN], F16)
            nc.vector.memset(cstate[:], 0.0)

            hs = {}  # step -> tile [H, NCN] (rotating, 3 live)
            with tc.tile_pool(name="hspool", bufs=1) as hsp, \
                 tc.tile_pool(name="psG", bufs=2, space="PSUM") as psG, \
                 tc.tile_pool(name="psY", bufs=2, space="PSUM") as psY, \
                 tc.tile_pool(name="wpL", bufs=2) as wpL:

                py_cur = [None]

                def emit_proj(t):
                    # y[:, t] = mdv * (hs[t].T @ wcomb); PSUM batched over 4
                    # steps (one [P, 640] tile = 2 banks; 64B regions never
                    # cross a bank boundary), drained once per batch.
                    tb = t % 4
                    if tb == 0:
                        py_cur[0] = psY.tile([P, NTILES * 64], F32,
                                             name="py", tag="py", bufs=1)
                    py = py_cur[0]
                    for k in range(NTILES):
                        rows = min(P, NCN - k * P)
                        o0 = k * 64 + tb * NF
                        nc.tensor.matmul(
                            out=py[:rows, o0:o0 + NF],
                            lhsT=hs[t][:, k * P:k * P + rows],
                            rhs=wcomb_sb[:], start=True, stop=True)
                    if tb == 3:
                        g0 = (t - 3) * NF
                        for k in range(NTILES):
                            rows = min(P, NCN - k * P)
                            nc.vector.tensor_scalar(
                                out=ysb[:rows, k * TNF + g0:k * TNF + g0 + 64],
                                in0=py[:rows, k * 64:(k + 1) * 64],
                                scalar1=mdv_sb[:rows, k:k + 1],
                                scalar2=None, op0=mybir.AluOpType.mult)

                for t in range(T):
                    prev = hdT if t == 0 else hs[t - 1]
                    sg = [None] * 4
                    for q in GATE_ORDER:
                        wsl = slice(q * H, (q + 1) * H)
                        pq = psG.tile([P, NCN], F32, tag="pq", bufs=2)
                        for off, sz in CH:
                            nc.tensor.matmul(out=pq[:, off:off + sz],
                                             lhsT=wih_sb[:, wsl],
                                             rhs=hdT[:, off:off + sz],
                                             start=True, stop=False)
                        for off, sz in CH:
                            nc.tensor.matmul(out=pq[:, off:off + sz],
                                             lhsT=whh_sb[:, wsl],
                                             rhs=prev[:, off:off + sz],
                                             start=False, stop=True)
                        sg[q] = wpL.tile([P, NCN], F16, name=f"sg{q}",
                                         tag=f"sg{q}", bufs=2)
                        nc.scalar.activation(
                            out=sg[q][:], in_=pq[:],
                            func=getattr(mybir.ActivationFunctionType,
                                         GATE_FUNCS[q]),
                            bias=bg_sb[:, q:q + 1])
                        if q == 1:  # f emitted -> c*f can start on DVE
                            cf = wpL.tile([P, NCN], F16, tag="cf", bufs=2)
                            nc.vector.tensor_mul(out=cf[:], in0=cstate[:],
                                                 in1=sg[1][:])
                    # c = f*c + i*g ; h = o * tanh(c)  (all fp16 on DVE)
                    ig = wpL.tile([P, NCN], F16, tag="ig", bufs=2)
                    nc.vector.tensor_mul(out=ig[:], in0=sg[0][:], in1=sg[2][:])
                    nc.vector.tensor_add(out=cstate[:], in0=cf[:], in1=ig[:])
                    thc = wpL.tile([P, NCN], F16, tag="thc", bufs=2)
                    nc.scalar.activation(
                        out=thc[:], in_=cstate[:],
                        func=mybir.ActivationFunctionType.Tanh)
                    h_t = hsp.tile([P, NCN], F16, tag="h", bufs=3)
                    nc.vector.tensor_mul(out=h_t[:], in0=sg[3][:], in1=thc[:])
                    hs[t] = h_t
                    # proj for t-1 fills the PE while ACT/DVE finish step t
                    if t > 0:
                        emit_proj(t - 1)
                emit_proj(T - 1)

            # ---- Y exchange: contiguous store -> AllGather -> ytab ----
            nc.sync.dma_start(yshard[:], ysb[:])
            nc.gpsimd.collective_compute(
                "AllGather", mybir.AluOpType.bypass,
                replica_groups=[list(range(NCORES))],
                ins=[yshard.opt()], outs=[yfull.opt()],
            )
            # ytab[p, (c*10+k)*192 + t*16+f] = Y[c*1250 + k*128 + p, t*16+f]
            ytab = sp.tile([P, NSB * TNF], F16, name="ytab")
            CW = NTILES * TNF  # 1920 cols per source core
            for c in range(NCORES):
                nc.sync.dma_start(ytab[:, c * CW:(c + 1) * CW],
                                  yfull[c * P:(c + 1) * P, :])

            # ---- GCN aggregation: pa[k] = sum_sb A[sb,k].T @ Y[sb] ----
            # A blocks stream in CH_SB-block fp8 chunks; all 10 dst tiles
            # accumulate in PSUM across the full sb sweep (single wave).
            with tc.tile_pool(name="psC", bufs=1, space="PSUM") as psC, \
                 tc.tile_pool(name="wpC", bufs=2) as wpC:
                # pair dst tiles 2-per-PSUM-bank: [128, 384] f32 = 1536B.
                # start=True clears the has_written bits of the WHOLE bank,
                # so paired regions can't each open their own accumulation
                # group: initialize each bank once with a K=1 zero-matmul
                # (sets has_written everywhere) and accumulate with
                # start=False throughout.
                pa2 = [psC.tile([P, 2 * TNF], F32, name=f"pa_{j}",
                                tag=f"pa{j}", bufs=1) for j in range(5)]
                pa = [pa2[k // 2][:, (k % 2) * TNF:(k % 2 + 1) * TNF]
                      for k in range(NTILES)]
                zrow = cp.tile([1, 2 * TNF], F16, name="zrow")
                nc.vector.memset(zrow[:], 0.0)
                for j in range(5):
                    nc.tensor.matmul(out=pa2[j][:], lhsT=zrow[:1, :P],
                                     rhs=zrow[:1, :], start=True, stop=False,
                                     skip_group_check=True)
                for s0 in range(0, NSB, CH_SB):
                    s1 = min(s0 + CH_SB, NSB)
                    nsb_c = s1 - s0
                    abc = wpC.tile([P, CH_SB * DCOL], F8E3, tag="abc",
                                   bufs=ABUFS)
                    nc.gpsimd.dma_start(
                        abc[:, :nsb_c * DCOL].rearrange(
                            "p (sb d) -> p sb d", d=DCOL),
                        ablk[s0 * P:s1 * P, :].rearrange(
                            "(sb p) d -> p sb d", p=P))
                    for sb in range(s0, s1):
                        srows = P if (sb % NTILES) != NTILES - 1 \
                            else NCN - (NTILES - 1) * P
                        aoff = (sb - s0) * DCOL
                        for k in range(NTILES):
                            nc.tensor.matmul(
                                out=pa[k][:],
                                lhsT=abc[:srows,
                                         aoff + k * P:aoff + (k + 1) * P],
                                rhs=ytab[:srows, sb * TNF:(sb + 1) * TNF],
                                start=False, stop=(sb == NSB - 1),
                                skip_group_check=True)
                # out = pa * dinv_dst + bias (fused), then store
                for k in range(NTILES):
                    rows = min(P, NCN - k * P)
                    osb = wpC.tile([P, TNF], F32, tag="osb", bufs=2)
                    nc.vector.scalar_tensor_tensor(
                        out=osb[:rows], in0=pa[k][:rows],
                        scalar=dinv_sb[:rows, k:k + 1],
                        in1=bout_sb[:rows],
                        op0=mybir.AluOpType.mult,
                        op1=mybir.AluOpType.add)
                    nc.sync.dma_start(xhat[k * P:k * P + rows, :],
                                      osb[:rows])

    nc.compile()
    return nc


def _preprocess(z, edge_index, x_mask, W_fc2, b_fc2, W_ih, W_hh, b_ih, b_hh,
                W_gcn, b_gcn, W_fc3, b_fc3):
    z = np.asarray(z, np.float32)
    edge_index = np.asarray(edge_index).astype(np.int64)
    x_mask = np.asarray(x_mask)
    src = edge_index[0]
    dst = edge_index[1]
    deg = (np.bincount(dst, minlength=N) + 1.0)
    dinv = (1.0 / np.sqrt(deg)).astype(np.float32)
    nmask = x_mask.reshape(N, -1).any(axis=1)
    mdv = dinv * nmask.astype(np.float32)

    src_all = np.concatenate([src, np.arange(N, dtype=np.int64)])
    dst_all = np.concatenate([dst, np.arange(N, dtype=np.int64)])

    # densify adjacency into per-core A blocks (per-core-local src blocking):
    # row (c_src*10 + k_src)*128 + p_src ; col k_dst*128 + p_dst
    sc = src_all // NCN
    sl = src_all % NCN
    sb = sc * NTILES + sl // P
    ps = sl % P
    core_of = dst_all // NCN
    dl = dst_all % NCN
    col = (dl // P) * P + dl % P  # == dl, but keep the tile structure explicit

    f8 = mybir.dt.np(F8E3)
    a_blocks = []
    lin = (sb * P + ps) * DCOL + col
    nblk_lin = NSB * P * DCOL
    for c in range(NCORES):
        m = core_of == c
        counts = np.bincount(lin[m], minlength=nblk_lin)
        assert counts.max() <= 15, "multiplicity overflows fp8-e3m4"
        a_blocks.append(counts.astype(f8).reshape(NSB * P, DCOL))

    Wcomb = np.ascontiguousarray((np.asarray(W_gcn, np.float32)
                                  @ np.asarray(W_fc3, np.float32))
                                 .astype(np.float16))
    bias16 = (np.asarray(b_gcn, np.float32) @ np.asarray(W_fc3, np.float32)
              + np.asarray(b_fc3, np.float32))
    bout_t = np.ascontiguousarray(np.tile(bias16, (P, T)).astype(np.float32))
    bgv = (np.asarray(b_ih, np.float32) + np.asarray(b_hh, np.float32))
    bg_t = np.ascontiguousarray(bgv.reshape(4, P).T.astype(np.float32))
    b2_t = np.ascontiguousarray(np.asarray(b_fc2, np.float32).reshape(P, 1))
    wih_t = np.ascontiguousarray(
        np.asarray(W_ih, np.float32).T.astype(np.float16))
    whh_t = np.ascontiguousarray(
        np.asarray(W_hh, np.float32).T.astype(np.float16))
    bf16 = ml_dtypes.bfloat16
    wfc2_t = np.ascontiguousarray(np.asarray(W_fc2, np.float32).astype(bf16))

    def per_node_tile(vec):
        out = np.zeros((P, NTILES), np.float32)
        for k in range(NTILES):
            rows = min(P, NCN - k * P)
            out[:rows, k] = vec[k * P:k * P + rows]
        return out

    in_maps = []
    for c in range(NCORES):
        slc = slice(c * NCN, (c + 1) * NCN)
        in_maps.append({
            "zT": np.ascontiguousarray(z[slc].T.astype(bf16)),
            "wfc2": wfc2_t,
            "b2": b2_t,
            "wih": wih_t,
            "whh": whh_t,
            "bg": bg_t,
            "wcomb": Wcomb,
            "bout": bout_t,
            "dinvt": per_node_tile(dinv[slc]),
            "mdvt": per_node_tile(mdv[slc]),
            "ablk": a_blocks[c],
        })
    return in_maps


def kernel(z, edge_index, x_mask, W_fc2, b_fc2, W_ih, W_hh, b_ih, b_hh,
           W_gcn, b_gcn, W_fc3, b_fc3):
    global LAST_RESULTS
    in_maps = _preprocess(z, edge_index, x_mask, W_fc2, b_fc2,
                          W_ih, W_hh, b_ih, b_hh,
                          W_gcn, b_gcn, W_fc3, b_fc3)
    if "nc" not in _BUILD_CACHE:
        _BUILD_CACHE["nc"] = _build()
    nc = _BUILD_CACHE["nc"]

    trace = bool(int(os.environ.get("KERNEL_TRACE", "0")))
    res = bass_utils.run_bass_kernel_spmd(
        nc, in_maps, core_ids=list(range(NCORES)), trace=trace)
    LAST_RESULTS = res

    out = np.empty((N, T, NF), np.float32)
    for c in range(NCORES):
        out[c * NCN:(c + 1) * NCN] = res.results[c]["xhat"].reshape(NCN, T, NF)
    return out


# revision 11
# speedup vs baseline: 1.2863x; 1.1313x over previous
# Trainium2 Bass kernel for nn_Decoder (LSTM decoder + GCN message passing).
#
# Strategy (8 NeuronCores, SPMD):
#   * Data-parallel over nodes N=10000 -> 1250 nodes/core for fc2 + LSTM +
#     projection. State kept feature-major ([H, nodes]) so every matmul is
#     PE-friendly with K=H=128 and no transposes.
#   * Algebraic rewrite: GCN aggregation and fc3 are both linear, so
#     aggregate AFTER projecting features to NF=16 (payload 8x smaller):
#        x_hat[n,t] = dinv[n] * sum_{e: dst=n} (dinv[s]*mask[s] * hs[t,s] @ (W_gcn@W_fc3))
#                     + (b_gcn@W_fc3 + b_fc3)
#   * The scatter-add over edges is a block-dense matmul: the host densifies
#     the normalized adjacency into 128x128 blocks (entry = multiplicity),
#     stored fp8-e3m4 (exact for small ints, halves HBM traffic); the PE
#     contracts them against the fp16 Y table (mixed-dtype matmul).
#   * Y table is built k-major so the AllGather input is one contiguous
#     store; a single AG at the end + 8 contiguous ytab loads (per-source-
#     core granularity) replaces the baseline's fragmented exchange.
#   * mask*dinv is computed on host (it only depends on inputs).
import os
import numpy as np
import ml_dtypes

import concourse.bass as bass
import concourse.bacc as bacc
import concourse.tile as tile
from concourse import mybir
from concourse import bass_utils

P = 128
N, T, NF, H, L, E = 10000, 12, 16, 128, 64, 160000
NCORES = 8
NCN = N // NCORES            # 1250 nodes per core
NTILES = (NCN + P - 1) // P  # 10 dst tiles per core (last has 98 rows)
NSB = NCORES * NTILES        # 80 source blocks (per-core-local blocking)
CH = [(0, 512), (512, 512), (1024, NCN - 1024)]  # matmul node chunks (<=512)
TNF = T * NF                 # 192
DCOL = NTILES * P            # 1280 A-block dst columns per core
CH_SB = 4                    # source blocks per streamed A chunk
ABUFS = 6                    # in-flight A chunks (SBUF prefetch depth)

F32 = mybir.dt.float32
F16 = mybir.dt.float16
BF16 = mybir.dt.bfloat16
F8E3 = mybir.dt.float8e3

# gate order within W_ih/W_hh: 0=i, 1=f, 2=g, 3=o
GATE_FUNCS = ["Sigmoid", "Sigmoid", "Tanh", "Sigmoid"]
GATE_ORDER = [1, 0, 2, 3]  # emit f first: starts the ACT/DVE chain earliest

_BUILD_CACHE = {}
LAST_RESULTS = None  # BassKernelResults of the most recent run (for test harness)


def _build():
    nc = bacc.Bacc("TRN2", target_bir_lowering=False, debug=False,
                   num_devices=NCORES)

    # ---------------- I/O declarations ----------------
    zT = nc.dram_tensor("zT", [L, NCN], BF16, kind="ExternalInput")
    wfc2 = nc.dram_tensor("wfc2", [L, H], BF16, kind="ExternalInput")
    b2 = nc.dram_tensor("b2", [P, 1], F32, kind="ExternalInput")
    wih = nc.dram_tensor("wih", [H, 4 * H], F16, kind="ExternalInput")
    whh = nc.dram_tensor("whh", [H, 4 * H], F16, kind="ExternalInput")
    bg = nc.dram_tensor("bg", [P, 4], F32, kind="ExternalInput")
    wcomb = nc.dram_tensor("wcomb", [H, NF], F16, kind="ExternalInput")
    bout = nc.dram_tensor("bout", [P, TNF], F32, kind="ExternalInput")
    dinvt = nc.dram_tensor("dinvt", [P, NTILES], F32, kind="ExternalInput")
    mdvt = nc.dram_tensor("mdvt", [P, NTILES], F32, kind="ExternalInput")
    # A-blocks: row sb*128 + src_rel ; col k*128 + dst_rel (sb = c*10 + k_src)
    ablk = nc.dram_tensor("ablk", [NSB * P, DCOL], F8E3, kind="ExternalInput")
    xhat = nc.dram_tensor("xhat", [NCN, TNF], F32, kind="ExternalOutput")

    with tile.TileContext(nc) as tc:
        with tc.tile_pool(name="cpool", bufs=1) as cp, \
             tc.tile_pool(name="spool", bufs=1) as sp, \
             tc.tile_pool(name="dram", bufs=1, space="DRAM") as dp:

            # ---- constant loads ----
            zt_sb = cp.tile([L, NCN], BF16)
            nc.sync.dma_start(zt_sb[:], zT[:])
            wfc2_sb = cp.tile([L, H], BF16)
            nc.sync.dma_start(wfc2_sb[:], wfc2[:])
            b2_sb = cp.tile([P, 1], F32)
            nc.sync.dma_start(b2_sb[:], b2[:])
            wih_sb = cp.tile([H, 4 * H], F16)
            nc.sync.dma_start(wih_sb[:], wih[:])
            whh_sb = cp.tile([H, 4 * H], F16)
            nc.sync.dma_start(whh_sb[:], whh[:])
            bg_sb = cp.tile([P, 4], F32)
            nc.sync.dma_start(bg_sb[:], bg[:])
            wcomb_sb = cp.tile([H, NF], F16)
            nc.sync.dma_start(wcomb_sb[:], wcomb[:])
            bout_sb = cp.tile([P, TNF], F32)
            nc.sync.dma_start(bout_sb[:], bout[:])
            dinv_sb = cp.tile([P, NTILES], F32)
            nc.sync.dma_start(dinv_sb[:], dinvt[:])
            mdv_sb = cp.tile([P, NTILES], F32)
            nc.sync.dma_start(mdv_sb[:], mdvt[:])

            # Y shard, split in 3 t-groups of 4 steps (AllGather pipelining):
            # ysb_g[p, k*64 + (t%4)*16 + f] holds Y[k*128+p, t] for t in
            # group g. Contiguous per-group stores/loads; 2 of 3 AGs hide
            # under the LSTM.
            NG = 3
            GW = 4 * NF  # 64 cols per group
            KW = NTILES * GW  # 640 cols per core per group
            ysb_g = [sp.tile([P, KW], F16, name=f"ysb{g}", tag=f"ysb{g}")
                     for g in range(NG)]
            for g in range(NG):
                nc.vector.memset(ysb_g[g][:], 0.0)  # rows >= 98 of tile 9
            yshard_g = [dp.tile([P, KW], F16, name=f"yshard{g}")
                        for g in range(NG)]
            yfull_g = [dp.tile([NCORES * P, KW], F16, addr_space="Shared",
                               name=f"yfull{g}") for g in range(NG)]
            # ytab_g[p, (c*10+k)*64 + (t%4)*16+f] = Y[c*1250+k*128+p, t]
            ytab_g = [sp.tile([P, NSB * GW], F16, name=f"ytab{g}",
                              tag=f"ytab{g}") for g in range(NG)]

            def ship_group(g):
                nc.sync.dma_start(yshard_g[g][:], ysb_g[g][:])
                nc.gpsimd.collective_compute(
                    "AllGather", mybir.AluOpType.bypass,
                    replica_groups=[list(range(NCORES))],
                    ins=[yshard_g[g].opt()], outs=[yfull_g[g].opt()],
                )
                for c in range(NCORES):
                    nc.sync.dma_start(ytab_g[g][:, c * KW:(c + 1) * KW],
                                      yfull_g[g][c * P:(c + 1) * P, :])

            # ---- hd = z @ W_fc2 + b_fc2 (feature-major: hdT [H, nodes]) ----
            hdT = sp.tile([H, NCN], F16)
            with tc.tile_pool(name="psI", bufs=2, space="PSUM") as psI:
                for off, sz in CH:
                    ph = psI.tile([P, 512], F32, tag="ph", bufs=2)
                    nc.tensor.matmul(out=ph[:, :sz], lhsT=wfc2_sb[:],
                                     rhs=zt_sb[:, off:off + sz],
                                     start=True, stop=True)
                    nc.scalar.activation(
                        out=hdT[:, off:off + sz], in_=ph[:, :sz],
                        func=mybir.ActivationFunctionType.Identity,
                        bias=b2_sb[:, :1])

            # ---- LSTM (T steps, feature-major state) ----
            cstate = sp.tile([P, NCN], F16)
            nc.vector.memset(cstate[:], 0.0)

            hs = {}  # step -> tile [H, NCN] (rotating, 3 live)
            with tc.tile_pool(name="hspool", bufs=1) as hsp, \
                 tc.tile_pool(name="psG", bufs=2, space="PSUM") as psG, \
                 tc.tile_pool(name="psY", bufs=2, space="PSUM") as psY, \
                 tc.tile_pool(name="wpL", bufs=2) as wpL:

                py_cur = [None]

                def emit_proj(t):
                    # y[:, t] = mdv * (hs[t].T @ wcomb); PSUM batched over 4
                    # steps (one [P, 640] tile = 2 banks; 64B regions never
                    # cross a bank boundary), drained once per batch.
                    tb = t % 4
                    if tb == 0:
                        py_cur[0] = psY.tile([P, NTILES * 64], F32,
                                             name="py", tag="py", bufs=1)
                    py = py_cur[0]
                    for k in range(NTILES):
                        rows = min(P, NCN - k * P)
                        o0 = k * 64 + tb * NF
                        nc.tensor.matmul(
                            out=py[:rows, o0:o0 + NF],
                            lhsT=hs[t][:, k * P:k * P + rows],
                            rhs=wcomb_sb[:], start=True, stop=True)
                    if tb == 3:
                        g = t // 4
                        for k in range(NTILES):
                            rows = min(P, NCN - k * P)
                            nc.vector.tensor_scalar(
                                out=ysb_g[g][:rows, k * GW:(k + 1) * GW],
                                in0=py[:rows, k * 64:(k + 1) * 64],
                                scalar1=mdv_sb[:rows, k:k + 1],
                                scalar2=None, op0=mybir.AluOpType.mult)
                        ship_group(g)

                for t in range(T):
                    prev = hdT if t == 0 else hs[t - 1]
                    sg = [None] * 4
                    for q in GATE_ORDER:
                        wsl = slice(q * H, (q + 1) * H)
                        pq = psG.tile([P, NCN], F32, tag="pq", bufs=2)
                        for off, sz in CH:
                            nc.tensor.matmul(out=pq[:, off:off + sz],
                                             lhsT=wih_sb[:, wsl],
                                             rhs=hdT[:, off:off + sz],
                                             start=True, stop=False)
                        for off, sz in CH:
                            nc.tensor.matmul(out=pq[:, off:off + sz],
                                             lhsT=whh_sb[:, wsl],
                                             rhs=prev[:, off:off + sz],
                                             start=False, stop=True)
                        sg[q] = wpL.tile([P, NCN], F16, name=f"sg{q}",
                                         tag=f"sg{q}", bufs=2)
                        nc.scalar.activation(
                            out=sg[q][:], in_=pq[:],
                            func=getattr(mybir.ActivationFunctionType,
                                         GATE_FUNCS[q]),
                            bias=bg_sb[:, q:q + 1])
                        if q == 1:  # f emitted -> c*f can start on DVE
                            cf = wpL.tile([P, NCN], F16, tag="cf", bufs=2)
                            nc.vector.tensor_mul(out=cf[:], in0=cstate[:],
                                                 in1=sg[1][:])
                    # c = f*c + i*g ; h = o * tanh(c)  (all fp16 on DVE)
                    ig = wpL.tile([P, NCN], F16, tag="ig", bufs=2)
                    nc.vector.tensor_mul(out=ig[:], in0=sg[0][:], in1=sg[2][:])
                    nc.vector.tensor_add(out=cstate[:], in0=cf[:], in1=ig[:])
                    thc = wpL.tile([P, NCN], F16, tag="thc", bufs=2)
                    nc.scalar.activation(
                        out=thc[:], in_=cstate[:],
                        func=mybir.ActivationFunctionType.Tanh)
                    h_t = hsp.tile([P, NCN], F16, tag="h", bufs=3)
                    nc.vector.tensor_mul(out=h_t[:], in0=sg[3][:], in1=thc[:])
                    hs[t] = h_t
                    # proj for t-1 fills the PE while ACT/DVE finish step t
                    if t > 0:
                        emit_proj(t - 1)
                emit_proj(T - 1)

            # ---- GCN aggregation: pa[k] = sum_sb A[sb,k].T @ Y[sb] ----
            # A blocks stream in CH_SB-block fp8 chunks; all 10 dst tiles
            # accumulate in PSUM across the full sb sweep (single wave).
            with tc.tile_pool(name="psC", bufs=1, space="PSUM") as psC, \
                 tc.tile_pool(name="wpC", bufs=2) as wpC:
                # pair dst tiles 2-per-PSUM-bank: [128, 384] f32 = 1536B.
                # start=True clears the has_written bits of the WHOLE bank,
                # so paired regions can't each open their own accumulation
                # group: initialize each bank once with a K=1 zero-matmul
                # (sets has_written everywhere) and accumulate with
                # start=False throughout.
                pa2 = [psC.tile([P, 2 * TNF], F32, name=f"pa_{j}",
                                tag=f"pa{j}", bufs=1) for j in range(5)]
                pa = [pa2[k // 2][:, (k % 2) * TNF:(k % 2 + 1) * TNF]
                      for k in range(NTILES)]
                zrow = cp.tile([1, 2 * TNF], F16, name="zrow")
                nc.vector.memset(zrow[:], 0.0)
                for j in range(5):
                    nc.tensor.matmul(out=pa2[j][:], lhsT=zrow[:1, :P],
                                     rhs=zrow[:1, :], start=True, stop=False,
                                     skip_group_check=True)
                for s0 in range(0, NSB, CH_SB):
                    s1 = min(s0 + CH_SB, NSB)
                    nsb_c = s1 - s0
                    abc = wpC.tile([P, CH_SB * DCOL], F8E3, tag="abc",
                                   bufs=ABUFS)
                    nc.gpsimd.dma_start(
                        abc[:, :nsb_c * DCOL].rearrange(
                            "p (sb d) -> p sb d", d=DCOL),
                        ablk[s0 * P:s1 * P, :].rearrange(
                            "(sb p) d -> p sb d", p=P))
                    for sb in range(s0, s1):
                        srows = P if (sb % NTILES) != NTILES - 1 \
                            else NCN - (NTILES - 1) * P
                        aoff = (sb - s0) * DCOL
                        for k in range(NTILES):
                            for g in range(NG):
                                nc.tensor.matmul(
                                    out=pa[k][:, g * GW:(g + 1) * GW],
                                    lhsT=abc[:srows,
                                             aoff + k * P:aoff + (k + 1) * P],
                                    rhs=ytab_g[g][:srows,
                                                  sb * GW:(sb + 1) * GW],
                                    start=False, stop=(sb == NSB - 1),
                                    skip_group_check=True)
                # out = pa * dinv_dst + bias (fused), then store
                for k in range(NTILES):
                    rows = min(P, NCN - k * P)
                    osb = wpC.tile([P, TNF], F32, tag="osb", bufs=2)
                    nc.vector.scalar_tensor_tensor(
                        out=osb[:rows], in0=pa[k][:rows],
                        scalar=dinv_sb[:rows, k:k + 1],
                        in1=bout_sb[:rows],
                        op0=mybir.AluOpType.mult,
                        op1=mybir.AluOpType.add)
                    nc.sync.dma_start(xhat[k * P:k * P + rows, :],
                                      osb[:rows])

    nc.compile()
    return nc


def _preprocess(z, edge_index, x_mask, W_fc2, b_fc2, W_ih, W_hh, b_ih, b_hh,
                W_gcn, b_gcn, W_fc3, b_fc3):
    z = np.asarray(z, np.float32)
    edge_index = np.asarray(edge_index).astype(np.int64)
    x_mask = np.asarray(x_mask)
    src = edge_index[0]
    dst = edge_index[1]
    deg = (np.bincount(dst, minlength=N) + 1.0)
    dinv = (1.0 / np.sqrt(deg)).astype(np.float32)
    nmask = x_mask.reshape(N, -1).any(axis=1)
    mdv = dinv * nmask.astype(np.float32)

    src_all = np.concatenate([src, np.arange(N, dtype=np.int64)])
    dst_all = np.concatenate([dst, np.arange(N, dtype=np.int64)])

    # densify adjacency into per-core A blocks (per-core-local src blocking):
    # row (c_src*10 + k_src)*128 + p_src ; col k_dst*128 + p_dst
    sc = src_all // NCN
    sl = src_all % NCN
    sb = sc * NTILES + sl // P
    ps = sl % P
    core_of = dst_all // NCN
    dl = dst_all % NCN
    col = (dl // P) * P + dl % P  # == dl, but keep the tile structure explicit

    f8 = mybir.dt.np(F8E3)
    a_blocks = []
    lin = (sb * P + ps) * DCOL + col
    nblk_lin = NSB * P * DCOL
    for c in range(NCORES):
        m = core_of == c
        counts = np.bincount(lin[m], minlength=nblk_lin)
        assert counts.max() <= 15, "multiplicity overflows fp8-e3m4"
        a_blocks.append(counts.astype(f8).reshape(NSB * P, DCOL))

    Wcomb = np.ascontiguousarray((np.asarray(W_gcn, np.float32)
                                  @ np.asarray(W_fc3, np.float32))
                                 .astype(np.float16))
    bias16 = (np.asarray(b_gcn, np.float32) @ np.asarray(W_fc3, np.float32)
              + np.asarray(b_fc3, np.float32))
    bout_t = np.ascontiguousarray(np.tile(bias16, (P, T)).astype(np.float32))
    bgv = (np.asarray(b_ih, np.float32) + np.asarray(b_hh, np.float32))
    bg_t = np.ascontiguousarray(bgv.reshape(4, P).T.astype(np.float32))
    b2_t = np.ascontiguousarray(np.asarray(b_fc2, np.float32).reshape(P, 1))
    wih_t = np.ascontiguousarray(
        np.asarray(W_ih, np.float32).T.astype(np.float16))
    whh_t = np.ascontiguousarray(
        np.asarray(W_hh, np.float32).T.astype(np.float16))
    bf16 = ml_dtypes.bfloat16
    wfc2_t = np.ascontiguousarray(np.asarray(W_fc2, np.float32).astype(bf16))

    def per_node_tile(vec):
        out = np.zeros((P, NTILES), np.float32)
        for k in range(NTILES):
            rows = min(P, NCN - k * P)
            out[:rows, k] = vec[k * P:k * P + rows]
        return out

    in_maps = []
    for c in range(NCORES):
        slc = slice(c * NCN, (c + 1) * NCN)
        in_maps.append({
            "zT": np.ascontiguousarray(z[slc].T.astype(bf16)),
            "wfc2": wfc2_t,
            "b2": b2_t,
            "wih": wih_t,
            "whh": whh_t,
            "bg": bg_t,
            "wcomb": Wcomb,
            "bout": bout_t,
            "dinvt": per_node_tile(dinv[slc]),
            "mdvt": per_node_tile(mdv[slc]),
            "ablk": a_blocks[c],
        })
    return in_maps


def kernel(z, edge_index, x_mask, W_fc2, b_fc2, W_ih, W_hh, b_ih, b_hh,
           W_gcn, b_gcn, W_fc3, b_fc3):
    global LAST_RESULTS
    in_maps = _preprocess(z, edge_index, x_mask, W_fc2, b_fc2,
                          W_ih, W_hh, b_ih, b_hh,
                          W_gcn, b_gcn, W_fc3, b_fc3)
    if "nc" not in _BUILD_CACHE:
        _BUILD_CACHE["nc"] = _build()
    nc = _BUILD_CACHE["nc"]

    trace = bool(int(os.environ.get("KERNEL_TRACE", "0")))
    res = bass_utils.run_bass_kernel_spmd(
        nc, in_maps, core_ids=list(range(NCORES)), trace=trace)
    LAST_RESULTS = res

    out = np.empty((N, T, NF), np.float32)
    for c in range(NCORES):
        out[c * NCN:(c + 1) * NCN] = res.results[c]["xhat"].reshape(NCN, T, NF)
    return out


# revision 14
# speedup vs baseline: 1.3449x; 1.0456x over previous
# Trainium2 Bass kernel for nn_Decoder (LSTM decoder + GCN message passing).
#
# Strategy (8 NeuronCores, SPMD):
#   * Data-parallel over nodes N=10000 -> 1250 nodes/core for fc2 + LSTM +
#     projection. State kept feature-major ([H, nodes]) so every matmul is
#     PE-friendly with K=H=128 and no transposes.
#   * Algebraic rewrite: GCN aggregation and fc3 are both linear, so
#     aggregate AFTER projecting features to NF=16 (payload 8x smaller):
#        x_hat[n,t] = dinv[n] * sum_{e: dst=n} (dinv[s]*mask[s] * hs[t,s] @ (W_gcn@W_fc3))
#                     + (b_gcn@W_fc3 + b_fc3)
#   * The scatter-add over edges is a block-dense matmul: the host densifies
#     the normalized adjacency into 128x128 blocks (entry = multiplicity),
#     stored fp8-e3m4 (exact for small ints, halves HBM traffic); the PE
#     contracts them against the fp16 Y table (mixed-dtype matmul).
#   * Y table is built k-major so the AllGather input is one contiguous
#     store; a single AG at the end + 8 contiguous ytab loads (per-source-
#     core granularity) replaces the baseline's fragmented exchange.
#   * mask*dinv is computed on host (it only depends on inputs).
import os
import numpy as np
import ml_dtypes

import concourse.bass as bass
import concourse.bacc as bacc
import concourse.tile as tile
from concourse import mybir
from concourse import bass_utils

P = 128
N, T, NF, H, L, E = 10000, 12, 16, 128, 64, 160000
NCORES = 8
NCN = N // NCORES            # 1250 nodes per core
NTILES = (NCN + P - 1) // P  # 10 dst tiles per core (last has 98 rows)
NSB = NCORES * NTILES        # 80 source blocks (per-core-local blocking)
CH = [(0, 512), (512, 512), (1024, NCN - 1024)]  # matmul node chunks (<=512)
TNF = T * NF                 # 192
DCOL = NTILES * P            # 1280 A-block dst columns per core
CH_SB = 4                    # source blocks per streamed A chunk
ABUFS = 4                    # in-flight A chunks (SBUF prefetch depth)

F32 = mybir.dt.float32
F16 = mybir.dt.float16
BF16 = mybir.dt.bfloat16
F8E3 = mybir.dt.float8e3

# gate order within W_ih/W_hh: 0=i, 1=f, 2=g, 3=o
GATE_FUNCS = ["Sigmoid", "Sigmoid", "Tanh", "Sigmoid"]
GATE_ORDER = [1, 0, 2, 3]  # emit f first: starts the ACT/DVE chain earliest

_BUILD_CACHE = {}
LAST_RESULTS = None  # BassKernelResults of the most recent run (for test harness)


def _build():
    nc = bacc.Bacc("TRN2", target_bir_lowering=False, debug=False,
                   num_devices=NCORES)

    # ---------------- I/O declarations ----------------
    zT = nc.dram_tensor("zT", [L, NCN], BF16, kind="ExternalInput")
    wfc2 = nc.dram_tensor("wfc2", [L, H], BF16, kind="ExternalInput")
    b2 = nc.dram_tensor("b2", [P, 1], F32, kind="ExternalInput")
    wih = nc.dram_tensor("wih", [H, 4 * H], F16, kind="ExternalInput")
    whh = nc.dram_tensor("whh", [H, 4 * H], F16, kind="ExternalInput")
    bg = nc.dram_tensor("bg", [P, 4], F32, kind="ExternalInput")
    wcomb = nc.dram_tensor("wcomb", [H, NF], F16, kind="ExternalInput")
    bout = nc.dram_tensor("bout", [P, TNF], F32, kind="ExternalInput")
    dinvt = nc.dram_tensor("dinvt", [P, NTILES], F32, kind="ExternalInput")
    mdvt = nc.dram_tensor("mdvt", [P, NTILES], F32, kind="ExternalInput")
    # A-blocks: row sb*128 + src_rel ; col k*128 + dst_rel (sb = c*10 + k_src)
    ablk = nc.dram_tensor("ablk", [NSB * P, DCOL], F8E3, kind="ExternalInput")
    xhat = nc.dram_tensor("xhat", [NCN, TNF], F32, kind="ExternalOutput")

    with tile.TileContext(nc) as tc:
        with tc.tile_pool(name="cpool", bufs=1) as cp, \
             tc.tile_pool(name="spool", bufs=1) as sp, \
             tc.tile_pool(name="dram", bufs=1, space="DRAM") as dp:

            # ---- constant loads ----
            zt_sb = cp.tile([L, NCN], BF16)
            nc.sync.dma_start(zt_sb[:], zT[:])
            wfc2_sb = cp.tile([L, H], BF16)
            nc.sync.dma_start(wfc2_sb[:], wfc2[:])
            b2_sb = cp.tile([P, 1], F32)
            nc.sync.dma_start(b2_sb[:], b2[:])
            wih_sb = cp.tile([H, 4 * H], F16)
            nc.sync.dma_start(wih_sb[:], wih[:])
            whh_sb = cp.tile([H, 4 * H], F16)
            nc.sync.dma_start(whh_sb[:], whh[:])
            bg_sb = cp.tile([P, 4], F32)
            nc.sync.dma_start(bg_sb[:], bg[:])
            wcomb_sb = cp.tile([H, NF], F16)
            nc.sync.dma_start(wcomb_sb[:], wcomb[:])
            bout_sb = cp.tile([P, TNF], F32)
            nc.sync.dma_start(bout_sb[:], bout[:])
            dinv_sb = cp.tile([P, NTILES], F32)
            nc.sync.dma_start(dinv_sb[:], dinvt[:])
            mdv_sb = cp.tile([P, NTILES], F32)
            nc.sync.dma_start(mdv_sb[:], mdvt[:])

            # Y shard, split in 3 t-groups of 4 steps (AllGather pipelining):
            # ysb_g[p, k*64 + (t%4)*16 + f] holds Y[k*128+p, t] for t in
            # group g. Contiguous per-group stores/loads; 2 of 3 AGs hide
            # under the LSTM.
            NG = 3
            GW = 4 * NF  # 64 cols per group
            KW = NTILES * GW  # 640 cols per core per group
            ysb_g = [sp.tile([P, KW], F16, name=f"ysb{g}", tag=f"ysb{g}")
                     for g in range(NG)]
            for g in range(NG):
                nc.vector.memset(ysb_g[g][:], 0.0)  # rows >= 98 of tile 9
            yshard_g = [dp.tile([P, KW], F16, name=f"yshard{g}")
                        for g in range(NG)]
            yfull_g = [dp.tile([NCORES * P, KW], F16, addr_space="Shared",
                               name=f"yfull{g}") for g in range(NG)]
            # ytab[p, (c*10+k)*192 + g*64 + u] = Y[c*1250+k*128+p, t=4g+u/16]
            # (t-interleaved per sb so the GCN runs ONE 192-col matmul per
            # block; group loads are strided 128B runs, hidden under LSTM
            # for g<2)
            ytab = sp.tile([P, NSB * TNF], F16, name="ytab")

            def ship_group(g):
                nc.sync.dma_start(yshard_g[g][:], ysb_g[g][:])
                nc.gpsimd.collective_compute(
                    "AllGather", mybir.AluOpType.bypass,
                    replica_groups=[list(range(NCORES))],
                    ins=[yshard_g[g].opt()], outs=[yfull_g[g].opt()],
                )
                for c in range(NCORES):
                    dst = ytab[:, c * NTILES * TNF:(c + 1) * NTILES * TNF] \
                        .rearrange("p (k gg u) -> p k gg u", k=NTILES,
                                   gg=NG)[:, :, g, :]
                    src = yfull_g[g][c * P:(c + 1) * P, :] \
                        .rearrange("p (k u) -> p k u", k=NTILES)
                    nc.sync.dma_start(dst, src)

            # ---- hd = z @ W_fc2 + b_fc2 (feature-major: hdT [H, nodes]) ----
            hdT = sp.tile([H, NCN], F16)
            with tc.tile_pool(name="psI", bufs=2, space="PSUM") as psI:
                for off, sz in CH:
                    ph = psI.tile([P, 512], F32, tag="ph", bufs=2)
                    nc.tensor.matmul(out=ph[:, :sz], lhsT=wfc2_sb[:],
                                     rhs=zt_sb[:, off:off + sz],
                                     start=True, stop=True)
                    nc.scalar.activation(
                        out=hdT[:, off:off + sz], in_=ph[:, :sz],
                        func=mybir.ActivationFunctionType.Identity,
                        bias=b2_sb[:, :1])

            # ---- LSTM (T steps, feature-major state) ----
            cstate = sp.tile([P, NCN], F16)
            nc.vector.memset(cstate[:], 0.0)

            hs = {}  # step -> tile [H, NCN] (rotating, 3 live)
            with tc.tile_pool(name="hspool", bufs=1) as hsp, \
                 tc.tile_pool(name="psG", bufs=2, space="PSUM") as psG, \
                 tc.tile_pool(name="psY", bufs=2, space="PSUM") as psY, \
                 tc.tile_pool(name="wpL", bufs=2) as wpL:

                py_cur = [None]

                def emit_proj(t):
                    # y[:, t] = mdv * (hs[t].T @ wcomb); PSUM batched over 4
                    # steps (one [P, 640] tile = 2 banks; 64B regions never
                    # cross a bank boundary), drained once per batch.
                    tb = t % 4
                    if tb == 0:
                        py_cur[0] = psY.tile([P, NTILES * 64], F32,
                                             name="py", tag="py", bufs=1)
                    py = py_cur[0]
                    for k in range(NTILES):
                        rows = min(P, NCN - k * P)
                        o0 = k * 64 + tb * NF
                        nc.tensor.matmul(
                            out=py[:rows, o0:o0 + NF],
                            lhsT=hs[t][:, k * P:k * P + rows],
                            rhs=wcomb_sb[:], start=True, stop=True)
                    if tb == 3:
                        g = t // 4
                        for k in range(NTILES):
                            rows = min(P, NCN - k * P)
                            nc.vector.tensor_scalar(
                                out=ysb_g[g][:rows, k * GW:(k + 1) * GW],
                                in0=py[:rows, k * 64:(k + 1) * 64],
                                scalar1=mdv_sb[:rows, k:k + 1],
                                scalar2=None, op0=mybir.AluOpType.mult)
                        ship_group(g)

                for t in range(T):
                    prev = hdT if t == 0 else hs[t - 1]
                    sg = [None] * 4
                    for q in GATE_ORDER:
                        wsl = slice(q * H, (q + 1) * H)
                        pq = psG.tile([P, NCN], F32, tag="pq", bufs=2)
                        for off, sz in CH:
                            nc.tensor.matmul(out=pq[:, off:off + sz],
                                             lhsT=wih_sb[:, wsl],
                                             rhs=hdT[:, off:off + sz],
                                             start=True, stop=False)
                        for off, sz in CH:
                            nc.tensor.matmul(out=pq[:, off:off + sz],
                                             lhsT=whh_sb[:, wsl],
                                             rhs=prev[:, off:off + sz],
                                             start=False, stop=True)
                        sg[q] = wpL.tile([P, NCN], F16, name=f"sg{q}",
                                         tag=f"sg{q}", bufs=2)
                        nc.scalar.activation(
                            out=sg[q][:], in_=pq[:],
                            func=getattr(mybir.ActivationFunctionType,
                                         GATE_FUNCS[q]),
                            bias=bg_sb[:, q:q + 1])
                        if q == 1:  # f emitted -> c*f can start on DVE
                            cf = wpL.tile([P, NCN], F16, tag="cf", bufs=2)
                            nc.vector.tensor_mul(out=cf[:], in0=cstate[:],
                                                 in1=sg[1][:])
                    # c = f*c + i*g ; h = o * tanh(c)  (all fp16 on DVE)
                    ig = wpL.tile([P, NCN], F16, tag="ig", bufs=2)
                    nc.vector.tensor_mul(out=ig[:], in0=sg[0][:], in1=sg[2][:])
                    nc.vector.tensor_add(out=cstate[:], in0=cf[:], in1=ig[:])
                    thc = wpL.tile([P, NCN], F16, tag="thc", bufs=2)
                    nc.scalar.activation(
                        out=thc[:], in_=cstate[:],
                        func=mybir.ActivationFunctionType.Tanh)
                    h_t = hsp.tile([P, NCN], F16, tag="h", bufs=3)
                    nc.vector.tensor_mul(out=h_t[:], in0=sg[3][:], in1=thc[:])
                    hs[t] = h_t
                    # proj for t-1 fills the PE while ACT/DVE finish step t
                    if t > 0:
                        emit_proj(t - 1)
                emit_proj(T - 1)

            # ---- GCN aggregation: pa[k] = sum_sb A[sb,k].T @ Y[sb] ----
            # A blocks stream in CH_SB-block fp8 chunks; all 10 dst tiles
            # accumulate in PSUM across the full sb sweep (single wave).
            with tc.tile_pool(name="psC", bufs=1, space="PSUM") as psC, \
                 tc.tile_pool(name="wpC", bufs=2) as wpC:
                # pair dst tiles 2-per-PSUM-bank: [128, 384] f32 = 1536B.
                # start=True clears the has_written bits of the WHOLE bank,
                # so paired regions can't each open their own accumulation
                # group: initialize each bank once with a K=1 zero-matmul
                # (sets has_written everywhere) and accumulate with
                # start=False throughout.
                pa2 = [psC.tile([P, 2 * TNF], F32, name=f"pa_{j}",
                                tag=f"pa{j}", bufs=1) for j in range(5)]
                pa = [pa2[k // 2][:, (k % 2) * TNF:(k % 2 + 1) * TNF]
                      for k in range(NTILES)]
                zrow = cp.tile([1, 2 * TNF], F16, name="zrow")
                nc.vector.memset(zrow[:], 0.0)
                for j in range(5):
                    nc.tensor.matmul(out=pa2[j][:], lhsT=zrow[:1, :P],
                                     rhs=zrow[:1, :], start=True, stop=False,
                                     skip_group_check=True)
                for s0 in range(0, NSB, CH_SB):
                    s1 = min(s0 + CH_SB, NSB)
                    nsb_c = s1 - s0
                    abc = wpC.tile([P, CH_SB * DCOL], F8E3, tag="abc",
                                   bufs=ABUFS)
                    nc.gpsimd.dma_start(
                        abc[:, :nsb_c * DCOL].rearrange(
                            "p (sb d) -> p sb d", d=DCOL),
                        ablk[s0 * P:s1 * P, :].rearrange(
                            "(sb p) d -> p sb d", p=P))
                    for sb in range(s0, s1):
                        srows = P if (sb % NTILES) != NTILES - 1 \
                            else NCN - (NTILES - 1) * P
                        aoff = (sb - s0) * DCOL
                        for k in range(NTILES):
                            nc.tensor.matmul(
                                out=pa[k][:],
                                lhsT=abc[:srows,
                                         aoff + k * P:aoff + (k + 1) * P],
                                rhs=ytab[:srows, sb * TNF:(sb + 1) * TNF],
                                start=False, stop=(sb == NSB - 1),
                                skip_group_check=True)
                # out = pa * dinv_dst + bias (fused), then store
                for k in range(NTILES):
                    rows = min(P, NCN - k * P)
                    osb = wpC.tile([P, TNF], F32, tag="osb", bufs=2)
                    nc.vector.scalar_tensor_tensor(
                        out=osb[:rows], in0=pa[k][:rows],
                        scalar=dinv_sb[:rows, k:k + 1],
                        in1=bout_sb[:rows],
                        op0=mybir.AluOpType.mult,
                        op1=mybir.AluOpType.add)
                    nc.sync.dma_start(xhat[k * P:k * P + rows, :],
                                      osb[:rows])

    nc.compile()
    return nc


def _preprocess(z, edge_index, x_mask, W_fc2, b_fc2, W_ih, W_hh, b_ih, b_hh,
                W_gcn, b_gcn, W_fc3, b_fc3):
    z = np.asarray(z, np.float32)
    edge_index = np.asarray(edge_index).astype(np.int64)
    x_mask = np.asarray(x_mask)
    src = edge_index[0]
    dst = edge_index[1]
    deg = (np.bincount(dst, minlength=N) + 1.0)
    dinv = (1.0 / np.sqrt(deg)).astype(np.float32)
    nmask = x_mask.reshape(N, -1).any(axis=1)
    mdv = dinv * nmask.astype(np.float32)

    src_all = np.concatenate([src, np.arange(N, dtype=np.int64)])
    dst_all = np.concatenate([dst, np.arange(N, dtype=np.int64)])

    # densify adjacency into per-core A blocks (per-core-local src blocking):
    # row (c_src*10 + k_src)*128 + p_src ; col k_dst*128 + p_dst
    sc = src_all // NCN
    sl = src_all % NCN
    sb = sc * NTILES + sl // P
    ps = sl % P
    core_of = dst_all // NCN
    dl = dst_all % NCN
    col = (dl // P) * P + dl % P  # == dl, but keep the tile structure explicit

    f8 = mybir.dt.np(F8E3)
    a_blocks = []
    lin = (sb * P + ps) * DCOL + col
    nblk_lin = NSB * P * DCOL
    for c in range(NCORES):
        m = core_of == c
        counts = np.bincount(lin[m], minlength=nblk_lin)
        assert counts.max() <= 15, "multiplicity overflows fp8-e3m4"
        a_blocks.append(counts.astype(f8).reshape(NSB * P, DCOL))

    Wcomb = np.ascontiguousarray((np.asarray(W_gcn, np.float32)
                                  @ np.asarray(W_fc3, np.float32))
                                 .astype(np.float16))
    bias16 = (np.asarray(b_gcn, np.float32) @ np.asarray(W_fc3, np.float32)
              + np.asarray(b_fc3, np.float32))
    bout_t = np.ascontiguousarray(np.tile(bias16, (P, T)).astype(np.float32))
    bgv = (np.asarray(b_ih, np.float32) + np.asarray(b_hh, np.float32))
    bg_t = np.ascontiguousarray(bgv.reshape(4, P).T.astype(np.float32))
    b2_t = np.ascontiguousarray(np.asarray(b_fc2, np.float32).reshape(P, 1))
    wih_t = np.ascontiguousarray(
        np.asarray(W_ih, np.float32).T.astype(np.float16))
    whh_t = np.ascontiguousarray(
        np.asarray(W_hh, np.float32).T.astype(np.float16))
    bf16 = ml_dtypes.bfloat16
    wfc2_t = np.ascontiguousarray(np.asarray(W_fc2, np.float32).astype(bf16))

    def per_node_tile(vec):
        out = np.zeros((P, NTILES), np.float32)
        for k in range(NTILES):
            rows = min(P, NCN - k * P)
            out[:rows, k] = vec[k * P:k * P + rows]
        return out

    in_maps = []
    for c in range(NCORES):
        slc = slice(c * NCN, (c + 1) * NCN)
        in_maps.append({
            "zT": np.ascontiguousarray(z[slc].T.astype(bf16)),
            "wfc2": wfc2_t,
            "b2": b2_t,
            "wih": wih_t,
            "whh": whh_t,
            "bg": bg_t,
            "wcomb": Wcomb,
            "bout": bout_t,
            "dinvt": per_node_tile(dinv[slc]),
            "mdvt": per_node_tile(mdv[slc]),
            "ablk": a_blocks[c],
        })
    return in_maps


def kernel(z, edge_index, x_mask, W_fc2, b_fc2, W_ih, W_hh, b_ih, b_hh,
           W_gcn, b_gcn, W_fc3, b_fc3):
    global LAST_RESULTS
    in_maps = _preprocess(z, edge_index, x_mask, W_fc2, b_fc2,
                          W_ih, W_hh, b_ih, b_hh,
                          W_gcn, b_gcn, W_fc3, b_fc3)
    if "nc" not in _BUILD_CACHE:
        _BUILD_CACHE["nc"] = _build()
    nc = _BUILD_CACHE["nc"]

    trace = bool(int(os.environ.get("KERNEL_TRACE", "0")))
    res = bass_utils.run_bass_kernel_spmd(
        nc, in_maps, core_ids=list(range(NCORES)), trace=trace)
    LAST_RESULTS = res

    out = np.empty((N, T, NF), np.float32)
    for c in range(NCORES):
        out[c * NCN:(c + 1) * NCN] = res.results[c]["xhat"].reshape(NCN, T, NF)
    return out
